# revision 50
# baseline (speedup 1.0000x reference)
"""COIL-style sparse-attention scoring kernel for Trainium2 (8 NeuronCores).

Reference computation:
    scores[q,i,d,j] = <query_tok_embs[q,i], doc_tok_embs[d,j]>         (K=32)
    masked = where(query_ids[q,i]==doc_ids[d,j], scores, 0)
    tok    = masked.max(axis=j)                                        (192 -> 1)
    tok_scores[q,d] = sum_i w[q,i] * tok[q,i,d]    (w drops CLS + SEP)
    out = tok_scores + query_cls_emb @ doc_cls_emb.T

Device strategy: data-parallel over the 64 queries (8 per core), doc side
replicated.  Key optimization: host-side candidate filtering — a doc
position (d,j) can only survive the exact-match mask for THIS core if its
token id appears among the core's <=256 query-token ids.  With a 5000-token
vocab that keeps ~10 of 192 positions per doc, so the cartesian score
matrix shrinks from [256, 24576] to [256, 1792] per core.

Layout per core (columns of one shared rhs; docs host-permuted so heavy
docs — >=SEG candidates, sorted by count — sit in slots 0..R2-1; the
inverse permutation is applied host-side to the output):
  * region1 [128*SEG=1536 cols = 3 PSUM banks]: first <=12 candidates of
    every doc -> ONE segmented reduce_max per 128-row block
  * region2 [64*SEG2=256 cols = 1 PSUM bank for both blocks]: overflow
    candidates of heavy docs (two slots for the <=8 super-heavy ones)
Masking folds into the contraction: token ids -> 5 base-6 digits -> 30
one-hot dims (0/1 doc side, C=128 query side) and a constant row adds
-5*128 = -640, so PSUM holds  aug = score + 128*(#match digits) - 640:
full matches land at score, partials <= -68, zero pad columns at exactly 0.
The final relu comes for free: light docs always have a zero pad column in
their segment, and heavy docs get max(...,0) folded into the region2 merge
(scalar_tensor_tensor).  This is exact: the reference masked-max is >= 0
since no (i,d) row matches all 192 positions, and |score|<60 (verified
host-side) keeps partial matches below any true match.  VectorE reduces
segments straight out of PSUM to bf16; CLS and per-token-weighted sums are
bf16 matmuls accumulating into one [8,128] PSUM tile.
"""

import numpy as np
from contextlib import ExitStack

import concourse.bass as bass
import concourse.bacc as bacc
import concourse.mybir as mybir
import concourse.tile as tile
from concourse.bass_utils import run_bass_kernel_spmd

F32 = mybir.dt.float32
BF16 = mybir.dt.bfloat16

# problem shape (hardcoded per contract)
BQ, LQ, BD, LD, TOK_D, CLS_D = 64, 32, 128, 192, 32, 768
NCORES = 8
QPC = BQ // NCORES          # 8 queries per core
NBLK = 2                    # two row-blocks of 128 = 4 queries x 32 tokens
ROWS = 128
DIG = 6                     # digit base; 6^5 = 7776 > 5000 vocab
NDIG = 5
KD = NDIG * DIG             # 30 one-hot dims
KC = TOK_D + KD + 1         # 63 = [d bf16; digit one-hots; offset row]
C = 128.0                   # per-digit match bonus
OFF = NDIG * C              # 640 full-match offset (folded into matmul)
SEG = 12                    # candidate slots per doc in region1
R2 = 48                     # heavy-doc (>=SEG cands) slots in region2
NS = 8                      # super-heavy (>=SEG+SEG2) second-slot count
SEG2 = 4                    # overflow slots per region2 slot
NSLOT = R2 + NS + 8         # 64 region2 slots (8 spare) = one PSUM bank for 2 blocks
N2 = NSLOT * SEG2           # 256 region2 columns
N1 = BD * SEG               # 1536 region1 columns = exactly 3 PSUM banks
NCOLS = N2 + N1             # 1792: rhs = [region2 | region1]
TN = 512                    # cols per matmul = one full PSUM bank
GRP = 3                     # region1 banks per block (one reduce per block)
AUXW = NBLK * QPC + 6 * QPC + 6 * BD  # sel | qclsT | dclsT packed [128, 832]


def build_nc():
    nc = bacc.Bacc(
        "TRN2",
        target_bir_lowering=False,
        debug=False,
        num_devices=NCORES,
    )

    # one [63, 256 region2 | 256 qlhsT | 2048 region1] tensor: fewer DMAs
    big_d = nc.dram_tensor("big", [KC, NBLK * ROWS + NCOLS], BF16, kind="ExternalInput")
    aux_d = nc.dram_tensor("aux", [128, AUXW], BF16, kind="ExternalInput")
    out_d = nc.dram_tensor("out", [QPC, BD], F32, kind="ExternalOutput")
    Q0 = N2                                   # qlhsT column origin inside big
    R0 = N2 + NBLK * ROWS                     # region1 column origin inside big

    with tile.TileContext(nc) as tc, ExitStack() as ctx:
        const = ctx.enter_context(tc.tile_pool(name="const", bufs=1))
        psum = ctx.enter_context(tc.tile_pool(name="psum", bufs=2, space="PSUM"))
        psum2 = ctx.enter_context(tc.tile_pool(name="psum2", bufs=1, space="PSUM"))
        opsum = ctx.enter_context(tc.tile_pool(name="opsum", bufs=1, space="PSUM"))
        work = ctx.enter_context(tc.tile_pool(name="work", bufs=1))

        big_t = const.tile([KC, NBLK * ROWS + NCOLS], BF16, tag="big")
        aux_t = const.tile([128, AUXW], BF16, tag="aux")
        r2rhs = big_t[:, 0:N2]
        qlhsT = big_t[:, Q0:R0]
        rhs_t = big_t[:, R0:R0 + N1]
        sel_t = aux_t[:, 0:NBLK * QPC]
        qclsT_t = aux_t[:, NBLK * QPC:NBLK * QPC + 6 * QPC]
        dclsT_t = aux_t[:, NBLK * QPC + 6 * QPC:AUXW]

        # small first chunk (region2 + qlhsT) alone on sync so the region2
        # matmuls and the DVE chain start as early as possible; region1 in two
        # wide chunks on the other two engines, split at the k0/k1 matmul
        # boundary; aux (CLS/sel, not critical-path) second on scalar
        cs = R0 + TN
        nc.sync.dma_start(big_t[:, 0:R0], big_d[:, 0:R0])
        nc.scalar.dma_start(big_t[:, R0:cs], big_d[:, R0:cs])
        nc.sync.dma_start(big_t[:, cs:cs + TN], big_d[:, cs:cs + TN])
        nc.gpsimd.dma_start(big_t[:, cs + TN:], big_d[:, cs + TN:])
        nc.scalar.dma_start(aux_t[:], aux_d[:])

        # --- region2 (heavy-doc overflow): one 384-col matmul per block;
        # the r2 PSUM slot is reused for out_ps afterwards ---
        tmp2 = work.tile([ROWS, NBLK * NSLOT], BF16, tag="tmp2")
        # both blocks' region2 scores share one PSUM bank
        r2ps = psum2.tile([128, NBLK, N2], F32, tag="r2")
        for b in range(NBLK):
            lhs = qlhsT[:, b * ROWS:(b + 1) * ROWS]
            nc.tensor.matmul(r2ps[:, b, :], lhs, r2rhs[:], start=True, stop=True)
        nc.vector.reduce_max(
            tmp2[:],
            r2ps[:, :, :].rearrange("p b (d s) -> p (b d) s", s=SEG2),
            axis=mybir.AxisListType.X,
        )

        # --- region1 score matmuls + one segmented max per block; the relu
        # comes from the guaranteed all-zero pad column of light docs and
        # from the max-with-0 in the heavy-doc merge ---
        merged = work.tile([ROWS, NBLK * R2], BF16, tag="merged")
        tokred = []
        for b in range(NBLK):
            tr = work.tile([ROWS, BD], BF16, tag=f"tokred{b}")
            lhs = qlhsT[:, b * ROWS:(b + 1) * ROWS]
            ps = psum.tile([128, GRP, TN], F32, tag="score")
            for k in range(GRP):
                nc.tensor.matmul(
                    ps[:, k, :], lhs,
                    rhs_t[:, k * TN:(k + 1) * TN],
                    start=True, stop=True,
                )
            flat = ps[:, :, :].rearrange("p g t -> p (g t)")
            if b == 0:
                # split at the 85-doc / 2-bank boundary: the first piece only
                # needs matmuls k0+k1, so it starts one matmul earlier (the
                # extra op's overhead sits in otherwise-idle DVE time)
                nc.vector.reduce_max(
                    tr[:, 0:85],
                    flat[:, 0:85 * SEG].rearrange("p (d s) -> p d s", s=SEG),
                    axis=mybir.AxisListType.X,
                )
                nc.vector.reduce_max(
                    tr[:, 85:BD],
                    flat[:, 85 * SEG:N1].rearrange("p (d s) -> p d s", s=SEG),
                    axis=mybir.AxisListType.X,
                )
            else:
                nc.vector.reduce_max(
                    tr[:],
                    flat.rearrange("p (d s) -> p d s", s=SEG),
                    axis=mybir.AxisListType.X,
                )
            # heavy docs: merged = max(relu(tr), overflow max).  Slot space:
            # [0:NS combine-dest | NS:R2 non-super extras | R2:R2+NS super
            # extras1 | R2+NS: super extras2]; pre-combining the super slots
            # into 0:NS makes slots 0:R2 line up with heavy docs 0:R2.
            t0 = b * NSLOT
            nc.vector.tensor_max(
                tmp2[:, t0:t0 + NS],
                tmp2[:, t0 + R2:t0 + R2 + NS],
                tmp2[:, t0 + R2 + NS:t0 + NSLOT],
            )
            nc.vector.scalar_tensor_tensor(
                merged[:, b * R2:(b + 1) * R2],
                tr[:, 0:R2],
                0.0,
                tmp2[:, t0:t0 + R2],
                op0=mybir.AluOpType.max,
                op1=mybir.AluOpType.max,
            )
            tokred.append(tr)

        out_ps = opsum.tile([QPC, BD], F32, tag="out_ps")

        # --- CLS matmuls (after score matmuls in PE order so their aux wait
        # can't clog the engine wait-queue), then weighted token sums ---
        for k in range(6):
            nc.tensor.matmul(
                out_ps[:],
                qclsT_t[:, k * QPC:(k + 1) * QPC],
                dclsT_t[:, k * BD:(k + 1) * BD],
                start=(k == 0),
                stop=False,
            )
        for b in range(NBLK):
            sel_b = sel_t[:, b * QPC:(b + 1) * QPC]
            nc.tensor.matmul(
                out_ps[:, 0:R2], sel_b, merged[:, b * R2:(b + 1) * R2],
                start=False, stop=(b == NBLK - 1),
            )
            nc.tensor.matmul(
                out_ps[:, R2:BD], sel_b, tokred[b][:, R2:BD],
                start=False, stop=(b == NBLK - 1),
            )

        outsb = work.tile([QPC, BD], F32, tag="outsb")
        nc.vector.tensor_copy(outsb[:], out_ps[:])
        nc.scalar.dma_start(out_d[:], outsb[:])

    nc.compile()
    return nc


_NC_CACHE = None


def _get_nc():
    global _NC_CACHE
    if _NC_CACHE is None:
        _NC_CACHE = build_nc()
    return _NC_CACHE


def _digits(ids):
    """ids [...] int -> [..., NDIG] base-6 digit values."""
    ids = np.asarray(ids, np.int64)
    return np.stack([(ids // (DIG ** t)) % DIG for t in range(NDIG)], axis=-1)


def _bf16(x):
    import ml_dtypes
    return np.ascontiguousarray(np.asarray(x, np.float32)).astype(ml_dtypes.bfloat16)


def make_in_maps(qte, dte, qce, dce, qid, did, qam):
    # SEP mask + CLS drop -> per-token weights
    sep = qam.sum(1) - 1
    qm = qam.astype(np.float32).copy()
    qm[np.arange(BQ), sep] = 0.0
    w = qm.copy()
    w[:, 0] = 0.0

    qdig = _digits(qid)                           # [64, 32, 5]
    ddig = _digits(did)                           # [128, 192, 5]

    in_maps = []
    perms = []
    for c in range(NCORES):
        qs = slice(c * QPC, (c + 1) * QPC)
        qte_c, qdig_c, w_c = qte[qs], qdig[qs], w[qs]

        # candidate filter: doc positions whose id appears in this core's set
        qids = np.unique(qid[qs])
        cand = np.isin(did, qids)                 # [128, 192]
        per_doc = cand.sum(1)
        heavy = np.nonzero(per_doc >= SEG)[0]
        heavy = heavy[np.argsort(-per_doc[heavy], kind="stable")]  # supers first
        nsuper = int((per_doc[heavy] > SEG + SEG2).sum())
        if heavy.size > R2:
            raise RuntimeError(f"core {c}: {heavy.size} heavy docs exceed R2={R2}")
        if nsuper > NS:
            raise RuntimeError(f"core {c}: {nsuper} super-heavy docs exceed NS={NS}")
        if per_doc.max() > SEG + 2 * SEG2:
            raise RuntimeError(f"core {c}: doc has {per_doc.max()} > {SEG+2*SEG2} cands")
        light = np.nonzero(per_doc < SEG)[0]
        perm = np.concatenate([heavy, light]).astype(np.int64)  # doc order used on device
        perms.append(perm)

        # rhs: [63, N2 + 128*SEG] bf16; pad columns stay fully zero (aug=0,
        # which folds the relu into the max)
        rhs = np.zeros((KC, NCOLS), np.float32)

        def fill(cols0, d, js):
            n = js.size
            if n == 0:
                return
            rhs[0:TOK_D, cols0:cols0 + n] = dte[d, js, :].T
            dg = ddig[d, js]
            for t in range(NDIG):
                rhs[TOK_D + t * DIG + dg[:, t], cols0 + np.arange(n)] = 1.0
            rhs[KC - 1, cols0:cols0 + n] = -OFF

        for s, d in enumerate(perm):
            js = np.nonzero(cand[d])[0]
            fill(N2 + s * SEG, d, js[:SEG])
            if js.size <= SEG:
                continue
            extras = js[SEG:]                     # s < R2 guaranteed (heavy first)
            if s < NS:                            # first NS heavy docs use the
                fill((R2 + s) * SEG2, d, extras[:SEG2])      # super two-slot path
                if extras.size > SEG2:
                    fill((R2 + NS + s) * SEG2, d, extras[SEG2:])
            else:
                fill(s * SEG2, d, extras)         # extras <= SEG2 by count sort

        qlhsT = np.zeros((KC, NBLK * ROWS), np.float32)
        for b in range(NBLK):
            blk = qte_c[b * 4:(b + 1) * 4].reshape(ROWS, TOK_D)
            qlhsT[0:TOK_D, b * ROWS:(b + 1) * ROWS] = blk.T
            dg = qdig_c[b * 4:(b + 1) * 4].reshape(ROWS, NDIG)
            for t in range(NDIG):
                qlhsT[TOK_D + t * DIG + dg[:, t], b * ROWS + np.arange(ROWS)] = C
        qlhsT[KC - 1, :] = 1.0

        sel = np.zeros((ROWS, NBLK * QPC), np.float32)
        for b in range(NBLK):
            for qq in range(4):
                ql_ = b * 4 + qq
                sel[qq * 32:(qq + 1) * 32, b * QPC + ql_] = w_c[ql_]

        qclsT = qce[qs].T.reshape(6, 128, QPC).transpose(1, 0, 2).reshape(128, 6 * QPC)
        # CLS doc columns must follow the same per-core doc permutation
        dclsT = dce[perm].T.reshape(6, 128, BD).transpose(1, 0, 2).reshape(128, 6 * BD)
        aux = np.concatenate([sel, qclsT, dclsT], axis=1)

        in_maps.append(
            {
                "big": _bf16(np.concatenate([rhs[:, 0:N2], qlhsT, rhs[:, N2:]], axis=1)),
                "aux": _bf16(aux),
            }
        )
    return in_maps, perms


def run(in_maps, trace=False, **kwargs):
    nc = _get_nc()
    return run_bass_kernel_spmd(
        nc, in_maps, core_ids=list(range(NCORES)), trace=trace, **kwargs
    )


def kernel(
    query_tok_embs,
    doc_tok_embs,
    query_cls_emb,
    doc_cls_emb,
    query_input_ids,
    doc_input_ids,
    query_attention_mask,
):
    qte = np.ascontiguousarray(np.asarray(query_tok_embs, np.float32))
    dte = np.ascontiguousarray(np.asarray(doc_tok_embs, np.float32))
    qce = np.ascontiguousarray(np.asarray(query_cls_emb, np.float32))
    dce = np.ascontiguousarray(np.asarray(doc_cls_emb, np.float32))
    qid = np.asarray(query_input_ids).astype(np.int64)
    did = np.asarray(doc_input_ids).astype(np.int64)
    qam = np.asarray(query_attention_mask).astype(np.int64)

    in_maps, perms = make_in_maps(qte, dte, qce, dce, qid, did, qam)
    res = run(in_maps)
    outs = []
    for c, r in enumerate(res.results):
        dev = np.asarray(r["out"], np.float32)    # [QPC, BD] in permuted doc order
        out = np.empty_like(dev)
        out[:, perms[c]] = dev                    # undo the doc permutation
        outs.append(out)
    return np.ascontiguousarray(np.concatenate(outs, axis=0).astype(np.float32))


# revision 52
# speedup vs baseline: 1.0343x; 1.0343x over previous
"""COIL-style sparse-attention scoring kernel for Trainium2 (8 NeuronCores).

Reference computation:
    scores[q,i,d,j] = <query_tok_embs[q,i], doc_tok_embs[d,j]>         (K=32)
    masked = where(query_ids[q,i]==doc_ids[d,j], scores, 0)
    tok    = masked.max(axis=j)                                        (192 -> 1)
    tok_scores[q,d] = sum_i w[q,i] * tok[q,i,d]    (w drops CLS + SEP)
    out = tok_scores + query_cls_emb @ doc_cls_emb.T

Device strategy: data-parallel over the 64 queries (8 per core), doc side
replicated.  Key optimization: host-side candidate filtering — a doc
position (d,j) can only survive the exact-match mask for THIS core if its
token id appears among the core's <=256 query-token ids.  With a 5000-token
vocab that keeps ~10 of 192 positions per doc, so the cartesian score
matrix shrinks from [256, 24576] to [256, 1792] per core.

Layout per core (columns of one shared rhs; docs host-permuted so heavy
docs — >=SEG candidates, sorted by count — sit in slots 0..R2-1; the
inverse permutation is applied host-side to the output):
  * region1 [128*SEG=1536 cols = 3 PSUM banks]: first <=12 candidates of
    every doc -> ONE segmented reduce_max per 128-row block
  * region2 [64*SEG2=256 cols = 1 PSUM bank for both blocks]: overflow
    candidates of heavy docs (two slots for the <=8 super-heavy ones)
Masking folds into the contraction: token ids -> 5 base-6 digits -> 30
one-hot dims (0/1 doc side, C=128 query side) and a constant row adds
-5*128 = -640, so PSUM holds  aug = score + 128*(#match digits) - 640:
full matches land at score, partials <= -68, zero pad columns at exactly 0.
The final relu comes for free: light docs always have a zero pad column in
their segment, and heavy docs get max(...,0) folded into the region2 merge
(scalar_tensor_tensor).  This is exact: the reference masked-max is >= 0
since no (i,d) row matches all 192 positions, and |score|<60 (verified
host-side) keeps partial matches below any true match.  VectorE reduces
segments straight out of PSUM to bf16; CLS and per-token-weighted sums are
bf16 matmuls accumulating into one [8,128] PSUM tile.
"""

import numpy as np
from contextlib import ExitStack

import concourse.bass as bass
import concourse.bacc as bacc
import concourse.mybir as mybir
import concourse.tile as tile
from concourse.bass_utils import run_bass_kernel_spmd

F32 = mybir.dt.float32
BF16 = mybir.dt.bfloat16

# problem shape (hardcoded per contract)
BQ, LQ, BD, LD, TOK_D, CLS_D = 64, 32, 128, 192, 32, 768
NCORES = 8
QPC = BQ // NCORES          # 8 queries per core
NBLK = 2                    # two row-blocks of 128 = 4 queries x 32 tokens
ROWS = 128
DIG = 6                     # digit base; 6^5 = 7776 > 5000 vocab
NDIG = 5
KD = NDIG * DIG             # 30 one-hot dims
KC = TOK_D + KD + 1         # 63 = [d bf16; digit one-hots; offset row]
C = 128.0                   # per-digit match bonus
OFF = NDIG * C              # 640 full-match offset (folded into matmul)
SEG = 12                    # candidate slots per doc in region1
R2 = 48                     # heavy-doc (>=SEG cands) slots in region2
NS = 8                      # super-heavy (>=SEG+SEG2) second-slot count
SEG2 = 4                    # overflow slots per region2 slot
NSLOT = R2 + NS + 8         # 64 region2 slots (8 spare) = one PSUM bank for 2 blocks
N2 = NSLOT * SEG2           # 256 region2 columns
N1 = BD * SEG               # 1536 region1 columns = exactly 3 PSUM banks
NCOLS = N2 + N1             # 1792: rhs = [region2 | region1]
TN = 512                    # cols per matmul = one full PSUM bank
GRP = 3                     # region1 banks per block (one reduce per block)
AUXW = NBLK * QPC + 6 * QPC + 6 * BD  # sel | qclsT | dclsT packed [128, 832]


def build_nc():
    nc = bacc.Bacc(
        "TRN2",
        target_bir_lowering=False,
        debug=False,
        num_devices=NCORES,
    )

    # one [63, 256 region2 | 256 qlhsT | 2048 region1] tensor: fewer DMAs
    big_d = nc.dram_tensor("big", [KC, NBLK * ROWS + NCOLS], BF16, kind="ExternalInput")
    aux_d = nc.dram_tensor("aux", [128, AUXW], BF16, kind="ExternalInput")
    out_d = nc.dram_tensor("out", [QPC, BD], F32, kind="ExternalOutput")
    Q0 = N2                                   # qlhsT column origin inside big
    R0 = N2 + NBLK * ROWS                     # region1 column origin inside big

    with tile.TileContext(nc) as tc, ExitStack() as ctx:
        const = ctx.enter_context(tc.tile_pool(name="const", bufs=1))
        psum = ctx.enter_context(tc.tile_pool(name="psum", bufs=2, space="PSUM"))
        psum2 = ctx.enter_context(tc.tile_pool(name="psum2", bufs=1, space="PSUM"))
        opsum = ctx.enter_context(tc.tile_pool(name="opsum", bufs=1, space="PSUM"))
        work = ctx.enter_context(tc.tile_pool(name="work", bufs=1))

        big_t = const.tile([KC, NBLK * ROWS + NCOLS], BF16, tag="big")
        aux_t = const.tile([128, AUXW], BF16, tag="aux")
        r2rhs = big_t[:, 0:N2]
        qlhsT = big_t[:, Q0:R0]
        rhs_t = big_t[:, R0:R0 + N1]
        sel_t = aux_t[:, 0:NBLK * QPC]
        qclsT_t = aux_t[:, NBLK * QPC:NBLK * QPC + 6 * QPC]
        dclsT_t = aux_t[:, NBLK * QPC + 6 * QPC:AUXW]

        # small first chunk (region2 + qlhsT) alone on sync so the region2
        # matmuls and the DVE chain start as early as possible; region1 in two
        # wide chunks on the other two engines, split at the k0/k1 matmul
        # boundary; aux (CLS/sel, not critical-path) second on scalar
        cs = R0 + TN
        nc.sync.dma_start(big_t[:, 0:R0], big_d[:, 0:R0])
        nc.scalar.dma_start(big_t[:, R0:cs], big_d[:, R0:cs])
        nc.sync.dma_start(big_t[:, cs:cs + TN], big_d[:, cs:cs + TN])
        nc.gpsimd.dma_start(big_t[:, cs + TN:], big_d[:, cs + TN:])
        nc.scalar.dma_start(aux_t[:], aux_d[:])

        # --- region2 (heavy-doc overflow): one 384-col matmul per block;
        # the r2 PSUM slot is reused for out_ps afterwards ---
        tmp2 = work.tile([ROWS, NBLK * NSLOT], BF16, tag="tmp2")
        # both blocks' region2 scores share one PSUM bank
        r2ps = psum2.tile([128, NBLK, N2], F32, tag="r2")
        for b in range(NBLK):
            lhs = qlhsT[:, b * ROWS:(b + 1) * ROWS]
            nc.tensor.matmul(r2ps[:, b, :], lhs, r2rhs[:], start=True, stop=True)
        nc.vector.reduce_max(
            tmp2[:],
            r2ps[:, :, :].rearrange("p b (d s) -> p (b d) s", s=SEG2),
            axis=mybir.AxisListType.X,
        )

        # --- region1 score matmuls + one segmented max per block; the relu
        # comes from the guaranteed all-zero pad column of light docs and
        # from the max-with-0 in the heavy-doc merge ---
        merged = work.tile([ROWS, NBLK * R2], BF16, tag="merged")
        tokred = []
        for b in range(NBLK):
            tr = work.tile([ROWS, BD], BF16, tag=f"tokred{b}")
            lhs = qlhsT[:, b * ROWS:(b + 1) * ROWS]
            ps = psum.tile([128, GRP, TN], F32, tag="score")
            for k in range(GRP):
                nc.tensor.matmul(
                    ps[:, k, :], lhs,
                    rhs_t[:, k * TN:(k + 1) * TN],
                    start=True, stop=True,
                )
            flat = ps[:, :, :].rearrange("p g t -> p (g t)")
            if b == 0:
                # split at the 85-doc / 2-bank boundary: the first piece only
                # needs matmuls k0+k1, so it starts one matmul earlier (the
                # extra op's overhead sits in otherwise-idle DVE time)
                nc.vector.reduce_max(
                    tr[:, 0:85],
                    flat[:, 0:85 * SEG].rearrange("p (d s) -> p d s", s=SEG),
                    axis=mybir.AxisListType.X,
                )
                nc.vector.reduce_max(
                    tr[:, 85:BD],
                    flat[:, 85 * SEG:N1].rearrange("p (d s) -> p d s", s=SEG),
                    axis=mybir.AxisListType.X,
                )
            else:
                nc.vector.reduce_max(
                    tr[:],
                    flat.rearrange("p (d s) -> p d s", s=SEG),
                    axis=mybir.AxisListType.X,
                )
            # heavy docs: merged = max(relu(tr), overflow max).  Slot space:
            # [0:NS combine-dest | NS:R2 non-super extras | R2:R2+NS super
            # extras1 | R2+NS: super extras2]; pre-combining the super slots
            # into 0:NS makes slots 0:R2 line up with heavy docs 0:R2.
            t0 = b * NSLOT
            nc.vector.tensor_max(
                tmp2[:, t0:t0 + NS],
                tmp2[:, t0 + R2:t0 + R2 + NS],
                tmp2[:, t0 + R2 + NS:t0 + NSLOT],
            )
            nc.vector.scalar_tensor_tensor(
                merged[:, b * R2:(b + 1) * R2],
                tr[:, 0:R2],
                0.0,
                tmp2[:, t0:t0 + R2],
                op0=mybir.AluOpType.max,
                op1=mybir.AluOpType.max,
            )
            tokred.append(tr)

        out_ps = opsum.tile([QPC, BD], F32, tag="out_ps")

        # --- CLS matmuls (after score matmuls in PE order so their aux wait
        # can't clog the engine wait-queue), then weighted token sums ---
        for k in range(6):
            nc.tensor.matmul(
                out_ps[:],
                qclsT_t[:, k * QPC:(k + 1) * QPC],
                dclsT_t[:, k * BD:(k + 1) * BD],
                start=(k == 0),
                stop=False,
            )
        for b in range(NBLK):
            sel_b = sel_t[:, b * QPC:(b + 1) * QPC]
            nc.tensor.matmul(
                out_ps[:, 0:R2], sel_b, merged[:, b * R2:(b + 1) * R2],
                start=False, stop=(b == NBLK - 1),
            )
            nc.tensor.matmul(
                out_ps[:, R2:BD], sel_b, tokred[b][:, R2:BD],
                start=False, stop=(b == NBLK - 1),
            )

        outsb = work.tile([QPC, BD], F32, tag="outsb")
        nc.vector.tensor_copy(outsb[:], out_ps[:])
        nc.scalar.dma_start(out_d[:], outsb[:])

    nc.compile()
    return nc


_NC_CACHE = None


def _get_nc():
    global _NC_CACHE
    if _NC_CACHE is None:
        _NC_CACHE = build_nc()
    return _NC_CACHE


def _digits(ids):
    """ids [...] int -> [..., NDIG] base-6 digit values."""
    ids = np.asarray(ids, np.int64)
    return np.stack([(ids // (DIG ** t)) % DIG for t in range(NDIG)], axis=-1)


def _bf16(x):
    import ml_dtypes
    return np.ascontiguousarray(np.asarray(x, np.float32)).astype(ml_dtypes.bfloat16)


def make_in_maps(qte, dte, qce, dce, qid, did, qam):
    # SEP mask + CLS drop -> per-token weights
    sep = qam.sum(1) - 1
    qm = qam.astype(np.float32).copy()
    qm[np.arange(BQ), sep] = 0.0
    w = qm.copy()
    w[:, 0] = 0.0

    qdig = _digits(qid)                           # [64, 32, 5]
    ddig = _digits(did)                           # [128, 192, 5]

    in_maps = []
    perms = []
    for c in range(NCORES):
        qs = slice(c * QPC, (c + 1) * QPC)
        qte_c, qdig_c, w_c = qte[qs], qdig[qs], w[qs]

        # candidate filter: doc positions whose id appears in this core's set
        qids = np.unique(qid[qs])
        cand = np.isin(did, qids)                 # [128, 192]
        per_doc = cand.sum(1)
        heavy = np.nonzero(per_doc >= SEG)[0]
        heavy = heavy[np.argsort(-per_doc[heavy], kind="stable")]  # supers first
        nsuper = int((per_doc[heavy] > SEG + SEG2).sum())
        if heavy.size > R2:
            raise RuntimeError(f"core {c}: {heavy.size} heavy docs exceed R2={R2}")
        if nsuper > NS:
            raise RuntimeError(f"core {c}: {nsuper} super-heavy docs exceed NS={NS}")
        if per_doc.max() > SEG + 2 * SEG2:
            raise RuntimeError(f"core {c}: doc has {per_doc.max()} > {SEG+2*SEG2} cands")
        light = np.nonzero(per_doc < SEG)[0]
        perm = np.concatenate([heavy, light]).astype(np.int64)  # doc order used on device
        perms.append(perm)

        # rhs: [63, N2 + 128*SEG] bf16; pad columns stay fully zero (aug=0,
        # which folds the relu into the max)
        rhs = np.zeros((KC, NCOLS), np.float32)

        def fill(cols0, d, js):
            n = js.size
            if n == 0:
                return
            rhs[0:TOK_D, cols0:cols0 + n] = dte[d, js, :].T
            dg = ddig[d, js]
            for t in range(NDIG):
                rhs[TOK_D + t * DIG + dg[:, t], cols0 + np.arange(n)] = 1.0
            rhs[KC - 1, cols0:cols0 + n] = -OFF

        for s, d in enumerate(perm):
            js = np.nonzero(cand[d])[0]
            fill(N2 + s * SEG, d, js[:SEG])
            if js.size <= SEG:
                continue
            extras = js[SEG:]                     # s < R2 guaranteed (heavy first)
            if s < NS:                            # first NS heavy docs use the
                fill((R2 + s) * SEG2, d, extras[:SEG2])      # super two-slot path
                if extras.size > SEG2:
                    fill((R2 + NS + s) * SEG2, d, extras[SEG2:])
            else:
                fill(s * SEG2, d, extras)         # extras <= SEG2 by count sort

        qlhsT = np.zeros((KC, NBLK * ROWS), np.float32)
        for b in range(NBLK):
            blk = qte_c[b * 4:(b + 1) * 4].reshape(ROWS, TOK_D)
            qlhsT[0:TOK_D, b * ROWS:(b + 1) * ROWS] = blk.T
            dg = qdig_c[b * 4:(b + 1) * 4].reshape(ROWS, NDIG)
            for t in range(NDIG):
                qlhsT[TOK_D + t * DIG + dg[:, t], b * ROWS + np.arange(ROWS)] = C
        qlhsT[KC - 1, :] = 1.0

        sel = np.zeros((ROWS, NBLK * QPC), np.float32)
        for b in range(NBLK):
            for qq in range(4):
                ql_ = b * 4 + qq
                sel[qq * 32:(qq + 1) * 32, b * QPC + ql_] = w_c[ql_]

        qclsT = qce[qs].T.reshape(6, 128, QPC).transpose(1, 0, 2).reshape(128, 6 * QPC)
        # CLS doc columns must follow the same per-core doc permutation
        dclsT = dce[perm].T.reshape(6, 128, BD).transpose(1, 0, 2).reshape(128, 6 * BD)
        aux = np.concatenate([sel, qclsT, dclsT], axis=1)

        in_maps.append(
            {
                "big": _bf16(np.concatenate([rhs[:, 0:N2], qlhsT, rhs[:, N2:]], axis=1)),
                "aux": _bf16(aux),
            }
        )
    return in_maps, perms


def run(in_maps, trace=False, **kwargs):
    nc = _get_nc()
    return run_bass_kernel_spmd(
        nc, in_maps, core_ids=list(range(NCORES)), trace=trace, **kwargs
    )


def kernel(
    query_tok_embs,
    doc_tok_embs,
    query_cls_emb,
    doc_cls_emb,
    query_input_ids,
    doc_input_ids,
    query_attention_mask,
):
    qte = np.ascontiguousarray(np.asarray(query_tok_embs, np.float32))
    dte = np.ascontiguousarray(np.asarray(doc_tok_embs, np.float32))
    qce = np.ascontiguousarray(np.asarray(query_cls_emb, np.float32))
    dce = np.ascontiguousarray(np.asarray(doc_cls_emb, np.float32))
    qid = np.asarray(query_input_ids).astype(np.int64)
    did = np.asarray(doc_input_ids).astype(np.int64)
    qam = np.asarray(query_attention_mask).astype(np.int64)

    in_maps, perms = make_in_maps(qte, dte, qce, dce, qid, did, qam)
    res = run(in_maps)
    outs = []
    for c, r in enumerate(res.results):
        dev = np.asarray(r["out"], np.float32)    # [QPC, BD] in permuted doc order
        out = np.empty_like(dev)
        out[:, perms[c]] = dev                    # undo the doc permutation
        outs.append(out)
    return np.ascontiguousarray(np.concatenate(outs, axis=0).astype(np.float32))
